# revision 35
# baseline (speedup 1.0000x reference)
"""Trainium2 Bass kernel for a Keras-style GRU (reset_after=True) + Dense(1) head.

Reference computation (per batch row):
    x_proj = x @ kernel + bias_i                      # [T, 3H]
    per step t:  hp = h @ rkernel + bias_r            # [3H]
        z  = sigmoid(xp[:H]      + hp[:H])
        r  = sigmoid(xp[H:2H]    + hp[H:2H])
        hh = tanh   (xp[2H:]     + r * hp[2H:])
        h  = z * h + (1 - z) * hh
    out = h_last @ dense_w + dense_b                  # [1]

Strategy (8 NeuronCores, data-parallel over batch, 64 rows/core):
  - Everything lives in a [gate/H on partitions, batch on free] layout so the
    recurrence needs no transposes.  x is transposed/cast to bf16 on the host.
  - z/r gates: the per-step input projection is accumulated DIRECTLY into the
    recurrence PSUM bank one step ahead: a K=4 indicator matmul injects the
    bias (start=True opens the bank), 16 N=64 matmuls add x_t @ wk, and the
    next step's recurrence matmuls accumulate h @ rkernel on top.  No
    identity-injection matmuls, no SBUF staging, no evacuations for z/r.
  - hh gate: x @ wk staged group-wise (8 steps, N=512 GEMM) two groups ahead
    since xp_h sits outside the r* product; bias_r[2H:] injected per step via
    the K=4 indicator matmul.
  - Recurrence matmuls run r-gate first, then z, then hr, so sigmoid(r)/
    sigmoid(z)/zc/a all execute under the matmul shadow; only the candidate
    tail (t1 -> t2 -> tanh -> b -> h) trails each step, split in halves and
    pipelined across DVE/ACT.
"""

import os
import sys

sys.path.insert(0, "/opt/trn_rl_repo")

import numpy as np
import ml_dtypes

import concourse.bass as bass
import concourse.mybir as mybir
import concourse.tile as tile
from concourse import bacc
from concourse.bass import ds
from concourse.bass_utils import run_bass_kernel_spmd

BF16 = mybir.dt.bfloat16
F32 = mybir.dt.float32
NP_BF16 = ml_dtypes.bfloat16

NCORES = 8
B, T, F, H = 512, 128, 512, 512
BS = B // NCORES          # 64 batch rows per core
G3 = 3 * H                # 1536
KF = F // 128             # 4 contraction chunks for x @ kernel
KH = H // 128             # 4 contraction chunks for h @ rkernel
NMH = KH                  # 4 hh-gate chunks of 128
GROUP = 8                 # timesteps per hh GEMM group
NB = GROUP * BS           # 512 free columns per hh GEMM group
HB = KH * BS // 2         # 128: half of the hidden free dim
AF = mybir.ActivationFunctionType
ALU = mybir.AluOpType


def build_program(n_steps=T):
    """Emit the full Bass/Tile program for one core."""
    n_groups = (n_steps + GROUP - 1) // GROUP
    nc = bacc.Bacc()

    # ---- DRAM parameters (per-core shapes; host pre-arranges layouts) ----
    xT = nc.declare_dram_parameter("xT", [F, T * BS], BF16, isOutput=False)
    wk = nc.declare_dram_parameter("wk", [F, G3], BF16, isOutput=False)
    wr = nc.declare_dram_parameter("wr", [H, G3], BF16, isOutput=False)
    # bias-injection lhsT tiles: row j holds the bias for gate chunk j; rows
    # 4..127 are zero so the matmul keeps the PE's (128,128) tile config (a
    # K=4 tile forces a config switch costing ~200ns on the next matmul).
    bz_l = nc.declare_dram_parameter("bz_l", [128, 128], BF16, isOutput=False)
    br_l = nc.declare_dram_parameter("br_l", [128, 128], BF16, isOutput=False)
    brh_l = nc.declare_dram_parameter("brh_l", [128, 128], BF16, isOutput=False)
    ind = nc.declare_dram_parameter("ind", [128, KH * BS], BF16, isOutput=False)
    # per-partition ACT bias for the hh GEMM evacuation (bias_i[2H:])
    biash = nc.declare_dram_parameter("biash", [128, NMH], F32, isOutput=False)
    wd = nc.declare_dram_parameter("wd", [128, KH], BF16, isOutput=False)
    db = nc.declare_dram_parameter("db", [1, 1], F32, isOutput=False)
    out = nc.declare_dram_parameter("out", [1, BS], F32, isOutput=True)

    xT_v = xT.ap().rearrange("(k p) n -> p k n", p=128)    # [128, KF, T*BS]
    wk_v = wk.ap().rearrange("(k p) g -> p k g", p=128)    # [128, KF, G3]
    wr_v = wr.ap().rearrange("(k p) g -> p k g", p=128)    # [128, KH, G3]

    with tile.TileContext(nc) as tc:
        with (
            tc.tile_pool(name="const", bufs=1) as p_const,
            tc.tile_pool(name="xt", bufs=4) as p_xt,
            tc.tile_pool(name="xp", bufs=3) as p_xp,
            tc.tile_pool(name="h", bufs=3) as p_h,
            tc.tile_pool(name="ew", bufs=2) as p_ew,
            tc.tile_pool(name="zps", bufs=2, space="PSUM") as p_z,
            tc.tile_pool(name="rps", bufs=1, space="PSUM") as p_r,
            tc.tile_pool(name="hrps", bufs=1, space="PSUM") as p_hr,
            tc.tile_pool(name="gps", bufs=2, space="PSUM") as p_g,
        ):
            # ---- resident constants ----
            wk_sb = p_const.tile([128, KF, G3], BF16)
            nc.sync.dma_start(out=wk_sb[:, :, :], in_=wk_v)
            wr_sb = p_const.tile([128, KH, G3], BF16)
            nc.sync.dma_start(out=wr_sb[:, :, :], in_=wr_v)
            bz_sb = p_const.tile([128, 128], BF16)
            nc.sync.dma_start(out=bz_sb[:, :], in_=bz_l.ap())
            br_sb = p_const.tile([128, 128], BF16)
            nc.sync.dma_start(out=br_sb[:, :], in_=br_l.ap())
            brh_sb = p_const.tile([128, 128], BF16)
            nc.sync.dma_start(out=brh_sb[:, :], in_=brh_l.ap())
            ind_sb = p_const.tile([128, KH * BS], BF16)
            nc.sync.dma_start(out=ind_sb[:, :], in_=ind.ap())
            biash_sb = p_const.tile([128, NMH], F32)
            nc.sync.dma_start(out=biash_sb[:, :], in_=biash.ap())
            wd_sb = p_const.tile([128, KH], BF16)
            nc.sync.dma_start(out=wd_sb[:, :], in_=wd.ap())
            db_sb = p_const.tile([1, 1], F32)
            nc.sync.dma_start(out=db_sb[:, :], in_=db.ap())

            # ---- xT group DMA ----
            xt_tiles = {}

            def emit_xt_dma(g):
                if g >= n_groups:
                    return
                t0 = p_xt.tile([128, KF, NB], BF16, name=f"xt{g}", tag="xt")
                nc.sync.dma_start(
                    out=t0[:, :, :], in_=xT_v[:, :, ds(g * NB, NB)]
                )
                xt_tiles[g] = t0

            # ---- hh-gate group GEMM (xp_h staged two groups ahead) ----
            xp_tiles = {}
            hh_ps = {}

            def emit_xp_alloc(g):
                if g >= n_groups:
                    return
                xp_tiles[g] = p_xp.tile(
                    [128, NMH, NB], BF16, name=f"xp{g}", tag="xp"
                )

            def emit_hh_mms(g, m):
                if g >= n_groups:
                    return
                ps = p_g.tile([128, NB], F32, name=f"gps{g}_{m}", tag="gps")
                hh_ps[(g, m)] = ps
                xt_t = xt_tiles[g]
                for k in range(KF):
                    nc.tensor.matmul(
                        out=ps[:, :],
                        lhsT=wk_sb[:, k, ds((2 * KH + m) * 128, 128)],
                        rhs=xt_t[:, k, :],
                        start=(k == 0),
                        stop=(k == KF - 1),
                    )

            def emit_hh_evac(g, m, engine):
                if g >= n_groups:
                    return
                ps = hh_ps.pop((g, m))
                dst = xp_tiles[g][:, m, :]
                b_ap = biash_sb[:, ds(m, 1)]
                if engine == "act":
                    nc.scalar.activation(dst, ps[:, :], AF.Identity, bias=b_ap)
                else:
                    nc.vector.tensor_scalar_add(out=dst, in0=ps[:, :], scalar1=b_ap)

            # ---- per-step z/r PSUM bank seeding: bias inject + x-proj fill ----
            banks = {}

            def emit_inject_fill(t):
                if t >= n_steps:
                    return
                g, tau = divmod(t, GROUP)
                z_ps = p_z.tile([128, KH * BS], F32, name=f"z{t}", tag="z")
                # r and hr live in half banks so sigmoid(r) / t1 start on
                # half A while the PE still accumulates half B.
                rA = p_r.tile([128, HB], F32, name=f"rA{t}", tag="rA")
                rB = p_r.tile([128, HB], F32, name=f"rB{t}", tag="rB")
                hrA = p_hr.tile([128, HB], F32, name=f"hrA{t}", tag="hrA")
                hrB = p_hr.tile([128, HB], F32, name=f"hrB{t}", tag="hrB")
                nc.tensor.matmul(
                    out=z_ps[:, :], lhsT=bz_sb[:, :], rhs=ind_sb[:, :],
                    start=True, stop=False,
                )
                xt_t = xt_tiles[g]
                for m in range(KH):
                    for k in range(KF):
                        nc.tensor.matmul(
                            out=z_ps[:, ds(m * BS, BS)],
                            lhsT=wk_sb[:, k, ds(m * 128, 128)],
                            rhs=xt_t[:, k, ds(tau * BS, BS)],
                            start=False, stop=False,
                            skip_group_check=True,
                        )
                # r/hr injections + fills last: those banks are
                # single-buffered, so they wait on the previous step's
                # sigmoid/t1 reads — long done by this point in the stream.
                for half, bank in ((0, rA), (1, rB)):
                    nc.tensor.matmul(
                        out=bank[:, :], lhsT=br_sb[:, :],
                        rhs=ind_sb[:, ds(half * HB, HB)],
                        start=True, stop=False,
                    )
                    for m in (2 * half, 2 * half + 1):
                        for k in range(KF):
                            nc.tensor.matmul(
                                out=bank[:, ds((m - 2 * half) * BS, BS)],
                                lhsT=wk_sb[:, k, ds((KH + m) * 128, 128)],
                                rhs=xt_t[:, k, ds(tau * BS, BS)],
                                start=False, stop=False,
                                skip_group_check=True,
                            )
                nc.tensor.matmul(
                    out=hrA[:, :], lhsT=brh_sb[:, :], rhs=ind_sb[:, 0:HB],
                    start=True, stop=False,
                )
                nc.tensor.matmul(
                    out=hrB[:, :], lhsT=brh_sb[:, :], rhs=ind_sb[:, HB:],
                    start=True, stop=False,
                )
                banks[t] = (z_ps, rA, rB, hrA, hrB)

            # ---- prologue ----
            for g in range(min(3, n_groups)):
                emit_xt_dma(g)
            for g in range(min(2, n_groups)):
                emit_xp_alloc(g)
                for m in range(NMH):
                    emit_hh_mms(g, m)
                    emit_hh_evac(g, m, "act" if m % 2 == 0 else "dve")
            emit_inject_fill(0)

            h_prev = p_h.tile([128, KH * BS], BF16, name="h_init", tag="h")
            nc.vector.memset(h_prev[:, :], 0.0)

            # ---- main loop ----
            for t in range(n_steps):
                g, tau = divmod(t, GROUP)
                z_ps, rA, rB, hrA, hrB = banks.pop(t)

                # recurrence matmuls.  r-halves first (each its own bank, so
                # sigmoid(r) half A fires after only 8 matmuls), then hr half
                # A (feeds t1 half 0), then z, then hr half B.
                def rec_half(bank, m_lo, gate):
                    for m in (m_lo, m_lo + 1):
                        for k in range(KH):
                            last = m == m_lo + 1 and k == KH - 1
                            nc.tensor.matmul(
                                out=bank[:, ds((m - m_lo) * BS, BS)],
                                lhsT=wr_sb[:, k, ds((gate * KH + m) * 128, 128)],
                                rhs=h_prev[:, ds(k * BS, BS)],
                                start=False, stop=last,
                                skip_group_check=not last,
                            )

                rsig = p_ew.tile([128, KH * BS], BF16, name=f"rs{t}", tag="rsig")
                rec_half(rA, 0, 1)
                nc.scalar.activation(rsig[:, 0:HB], rA[:, :], AF.Sigmoid)
                rec_half(rB, 2, 1)
                nc.scalar.activation(rsig[:, HB:], rB[:, :], AF.Sigmoid)

                rec_half(hrA, 0, 2)

                for kk in (0, 2):
                    for m in range(KH):
                        for k in (kk, kk + 1):
                            last = kk == 2 and m == KH - 1 and k == KH - 1
                            nc.tensor.matmul(
                                out=z_ps[:, ds(m * BS, BS)],
                                lhsT=wr_sb[:, k, ds(m * 128, 128)],
                                rhs=h_prev[:, ds(k * BS, BS)],
                                start=False, stop=last,
                                skip_group_check=not last,
                            )
                zsig = p_ew.tile([128, KH * BS], BF16, name=f"zs{t}", tag="zsig")
                nc.scalar.activation(zsig[:, :], z_ps[:, :], AF.Sigmoid)
                zc = p_ew.tile([128, KH * BS], BF16, name=f"zc{t}", tag="zc")
                nc.gpsimd.tensor_scalar(
                    out=zc[:, :], in0=zsig[:, :], scalar1=-1.0, scalar2=1.0,
                    op0=ALU.mult, op1=ALU.add,
                )
                a_t = p_ew.tile([128, KH * BS], BF16, name=f"a{t}", tag="a")
                nc.gpsimd.tensor_mul(a_t[:, :], zsig[:, :], h_prev[:, :])

                rec_half(hrB, 2, 2)

                # candidate tail: t1 = hr*r, t2 = t1+xp_h, hh = tanh(t2),
                # b = hh*zc, h = a+b.  Halves pipelined; DVE FIFO ordered to
                # avoid head-of-line blocking on the ACT tanh.
                xp_g = xp_tiles[g]
                t1h, t2h = [], []
                for half, hr_bank in ((0, hrA), (1, hrB)):
                    sl = ds(half * HB, HB)
                    t1 = p_ew.tile([128, HB], F32, name=f"t1_{t}_{half}", tag=f"t1{half}")
                    nc.vector.tensor_tensor(
                        out=t1[:, :], in0=hr_bank[:, :], in1=rsig[:, sl], op=ALU.mult
                    )
                    t2 = p_ew.tile([128, 2, BS], F32, name=f"t2_{t}_{half}", tag=f"t2{half}")
                    nc.vector.tensor_tensor(
                        out=t2[:, :, :],
                        in0=t1[:, :].rearrange("p (m b) -> p m b", b=BS),
                        in1=xp_g[:, ds(2 * half, 2), ds(tau * BS, BS)],
                        op=ALU.add,
                    )
                    t1h.append(t1)
                    t2h.append(t2)
                hhh = []
                for half in range(2):
                    hh = p_ew.tile([128, HB], BF16, name=f"hh{t}_{half}", tag=f"hh{half}")
                    nc.scalar.activation(
                        hh[:, :], t2h[half][:, :, :].rearrange("p m b -> p (m b)"),
                        AF.Tanh,
                    )
                    hhh.append(hh)
                h_new = p_h.tile([128, KH * BS], BF16, name=f"h{t}", tag="h")
                for half in range(2):
                    sl = ds(half * HB, HB)
                    b_t = p_ew.tile([128, HB], BF16, name=f"b{t}_{half}", tag=f"b{half}")
                    nc.vector.tensor_tensor(
                        out=b_t[:, :], in0=hhh[half][:, :], in1=zc[:, sl], op=ALU.mult
                    )
                    nc.vector.tensor_tensor(
                        out=h_new[:, sl], in0=a_t[:, sl], in1=b_t[:, :], op=ALU.add
                    )
                h_prev = h_new

                # next step's bank seeding + amortized hh GEMM + DMA
                if tau == 0:
                    emit_xt_dma(g + 3)
                    emit_xp_alloc(g + 2)
                emit_inject_fill(t + 1)
                if tau % 2 == 0:
                    emit_hh_mms(g + 2, tau // 2)
                else:
                    emit_hh_evac(g + 2, tau // 2, "act" if tau % 4 == 1 else "dve")

            # ---- dense head: out = h_last @ dense_w + dense_b ----
            d_ps = p_g.tile([1, BS], F32, name="dense_ps", tag="gps")
            for k in range(KH):
                nc.tensor.matmul(
                    out=d_ps[0:1, :],
                    lhsT=wd_sb[:, ds(k, 1)],
                    rhs=h_prev[:, ds(k * BS, BS)],
                    start=(k == 0),
                    stop=(k == KH - 1),
                )
            out_sb = p_const.tile([1, BS], F32)
            nc.scalar.activation(
                out_sb[0:1, :], d_ps[0:1, :], AF.Identity, bias=db_sb[0:1, 0:1]
            )
            nc.sync.dma_start(out=out.ap(), in_=out_sb[0:1, :])

    nc.finalize()
    return nc


def prep_inputs(x, kernel, rkernel, bias_i, bias_r, dense_w, dense_b, n_steps=T):
    """Host-side shard + layout prep. Returns in_maps for run_bass_kernel_spmd."""
    x = np.asarray(x, dtype=np.float32)
    kernel = np.asarray(kernel, dtype=np.float32)
    rkernel = np.asarray(rkernel, dtype=np.float32)
    bias_i = np.asarray(bias_i, dtype=np.float32)
    bias_r = np.asarray(bias_r, dtype=np.float32)
    dense_w = np.asarray(dense_w, dtype=np.float32)
    dense_b = np.asarray(dense_b, dtype=np.float32)

    wk_h = np.ascontiguousarray(kernel.astype(NP_BF16))
    wr_h = np.ascontiguousarray(rkernel.astype(NP_BF16))
    comb = bias_i[: 2 * H] + bias_r[: 2 * H]

    def pad_bias(v):
        m = np.zeros((128, 128), dtype=NP_BF16)
        m[:KH] = v.reshape(KH, 128).astype(NP_BF16)
        return np.ascontiguousarray(m)

    bz_h = pad_bias(comb[:H])
    br_h = pad_bias(comb[H:])
    brh_h = pad_bias(bias_r[2 * H:])
    ind_h = np.zeros((128, KH * BS), dtype=NP_BF16)
    for j in range(KH):
        ind_h[j, j * BS:(j + 1) * BS] = 1
    biash_h = np.ascontiguousarray(
        bias_i[2 * H:].reshape(NMH, 128).T.astype(np.float32)
    )
    wd_h = np.ascontiguousarray(dense_w.reshape(KH, 128).T.astype(NP_BF16))
    db_h = dense_b.reshape(1, 1).astype(np.float32)

    in_maps = []
    for c in range(NCORES):
        xs = x[c * BS:(c + 1) * BS]                       # [BS, T, F]
        xT_h = np.ascontiguousarray(
            xs.transpose(2, 1, 0).reshape(F, T * BS).astype(NP_BF16)
        )
        in_maps.append(
            {
                "xT": xT_h,
                "wk": wk_h,
                "wr": wr_h,
                "bz_l": bz_h,
                "br_l": br_h,
                "brh_l": brh_h,
                "ind": ind_h,
                "biash": biash_h,
                "wd": wd_h,
                "db": db_h,
            }
        )
    return in_maps


def kernel(x, kernel, rkernel, bias_i, bias_r, dense_w, dense_b):
    nc = build_program()
    in_maps = prep_inputs(x, kernel, rkernel, bias_i, bias_r, dense_w, dense_b)
    res = run_bass_kernel_spmd(nc, in_maps, list(range(NCORES)))
    outs = [res.results[i]["out"].reshape(BS, 1) for i in range(NCORES)]
    return np.concatenate(outs, axis=0).astype(np.float32)


# revision 38
# speedup vs baseline: 1.0274x; 1.0274x over previous
"""Trainium2 Bass kernel for a Keras-style GRU (reset_after=True) + Dense(1) head.

Reference computation (per batch row):
    x_proj = x @ kernel + bias_i                      # [T, 3H]
    per step t:  hp = h @ rkernel + bias_r            # [3H]
        z  = sigmoid(xp[:H]      + hp[:H])
        r  = sigmoid(xp[H:2H]    + hp[H:2H])
        hh = tanh   (xp[2H:]     + r * hp[2H:])
        h  = z * h + (1 - z) * hh
    out = h_last @ dense_w + dense_b                  # [1]

Strategy (8 NeuronCores, data-parallel over batch, 64 rows/core):
  - Everything lives in a [gate/H on partitions, batch on free] layout so the
    recurrence needs no transposes.  x is transposed/cast to bf16 on the host.
  - z/r gates: the per-step input projection is accumulated DIRECTLY into the
    recurrence PSUM bank one step ahead: a K=4 indicator matmul injects the
    bias (start=True opens the bank), 16 N=64 matmuls add x_t @ wk, and the
    next step's recurrence matmuls accumulate h @ rkernel on top.  No
    identity-injection matmuls, no SBUF staging, no evacuations for z/r.
  - hh gate: x @ wk staged group-wise (8 steps, N=512 GEMM) two groups ahead
    since xp_h sits outside the r* product; bias_r[2H:] injected per step via
    the K=4 indicator matmul.
  - Recurrence matmuls run r-gate first, then z, then hr, so sigmoid(r)/
    sigmoid(z)/zc/a all execute under the matmul shadow; only the candidate
    tail (t1 -> t2 -> tanh -> b -> h) trails each step, split in halves and
    pipelined across DVE/ACT.
"""

import os
import sys

sys.path.insert(0, "/opt/trn_rl_repo")

import numpy as np
import ml_dtypes

import concourse.bass as bass
import concourse.mybir as mybir
import concourse.tile as tile
from concourse import bacc
from concourse.bass import ds
from concourse.bass_utils import run_bass_kernel_spmd

BF16 = mybir.dt.bfloat16
F32 = mybir.dt.float32
NP_BF16 = ml_dtypes.bfloat16

NCORES = 8
B, T, F, H = 512, 128, 512, 512
BS = B // NCORES          # 64 batch rows per core
G3 = 3 * H                # 1536
KF = F // 128             # 4 contraction chunks for x @ kernel
KH = H // 128             # 4 contraction chunks for h @ rkernel
NMH = KH                  # 4 hh-gate chunks of 128
GROUP = 8                 # timesteps per hh GEMM group
NB = GROUP * BS           # 512 free columns per hh GEMM group
HB = KH * BS // 2         # 128: half of the hidden free dim
AF = mybir.ActivationFunctionType
ALU = mybir.AluOpType


def build_program(n_steps=T):
    """Emit the full Bass/Tile program for one core."""
    n_groups = (n_steps + GROUP - 1) // GROUP
    nc = bacc.Bacc()

    # ---- DRAM parameters (per-core shapes; host pre-arranges layouts) ----
    xT = nc.declare_dram_parameter("xT", [F, T * BS], BF16, isOutput=False)
    wk = nc.declare_dram_parameter("wk", [F, G3], BF16, isOutput=False)
    wr = nc.declare_dram_parameter("wr", [H, G3], BF16, isOutput=False)
    # bias-injection lhsT tiles: row j holds the bias for gate chunk j; rows
    # 4..127 are zero so the matmul keeps the PE's (128,128) tile config (a
    # K=4 tile forces a config switch costing ~200ns on the next matmul).
    bz_l = nc.declare_dram_parameter("bz_l", [128, 128], BF16, isOutput=False)
    br_l = nc.declare_dram_parameter("br_l", [128, 128], BF16, isOutput=False)
    brh_l = nc.declare_dram_parameter("brh_l", [128, 128], BF16, isOutput=False)
    ind = nc.declare_dram_parameter("ind", [128, KH * BS], BF16, isOutput=False)
    # per-partition ACT bias for the hh GEMM evacuation (bias_i[2H:])
    biash = nc.declare_dram_parameter("biash", [128, NMH], F32, isOutput=False)
    wd = nc.declare_dram_parameter("wd", [128, KH], BF16, isOutput=False)
    db = nc.declare_dram_parameter("db", [1, 1], F32, isOutput=False)
    out = nc.declare_dram_parameter("out", [1, BS], F32, isOutput=True)

    xT_v = xT.ap().rearrange("(k p) n -> p k n", p=128)    # [128, KF, T*BS]
    wk_v = wk.ap().rearrange("(k p) g -> p k g", p=128)    # [128, KF, G3]
    wr_v = wr.ap().rearrange("(k p) g -> p k g", p=128)    # [128, KH, G3]

    with tile.TileContext(nc) as tc:
        with (
            tc.tile_pool(name="const", bufs=1) as p_const,
            tc.tile_pool(name="xt", bufs=4) as p_xt,
            tc.tile_pool(name="xp", bufs=3) as p_xp,
            tc.tile_pool(name="h", bufs=3) as p_h,
            tc.tile_pool(name="ew", bufs=2) as p_ew,
            tc.tile_pool(name="zps", bufs=2, space="PSUM") as p_z,
            tc.tile_pool(name="rps", bufs=2, space="PSUM") as p_r,
            tc.tile_pool(name="hrps", bufs=1, space="PSUM") as p_hr,
            tc.tile_pool(name="gps", bufs=2, space="PSUM") as p_g,
        ):
            # ---- resident constants ----
            wk_sb = p_const.tile([128, KF, G3], BF16)
            nc.sync.dma_start(out=wk_sb[:, :, :], in_=wk_v)
            wr_sb = p_const.tile([128, KH, G3], BF16)
            nc.sync.dma_start(out=wr_sb[:, :, :], in_=wr_v)
            bz_sb = p_const.tile([128, 128], BF16)
            nc.sync.dma_start(out=bz_sb[:, :], in_=bz_l.ap())
            br_sb = p_const.tile([128, 128], BF16)
            nc.sync.dma_start(out=br_sb[:, :], in_=br_l.ap())
            brh_sb = p_const.tile([128, 128], BF16)
            nc.sync.dma_start(out=brh_sb[:, :], in_=brh_l.ap())
            ind_sb = p_const.tile([128, KH * BS], BF16)
            nc.sync.dma_start(out=ind_sb[:, :], in_=ind.ap())
            biash_sb = p_const.tile([128, NMH], F32)
            nc.sync.dma_start(out=biash_sb[:, :], in_=biash.ap())
            wd_sb = p_const.tile([128, KH], BF16)
            nc.sync.dma_start(out=wd_sb[:, :], in_=wd.ap())
            db_sb = p_const.tile([1, 1], F32)
            nc.sync.dma_start(out=db_sb[:, :], in_=db.ap())

            # ---- xT group DMA ----
            xt_tiles = {}

            def emit_xt_dma(g):
                if g >= n_groups:
                    return
                t0 = p_xt.tile([128, KF, NB], BF16, name=f"xt{g}", tag="xt")
                nc.sync.dma_start(
                    out=t0[:, :, :], in_=xT_v[:, :, ds(g * NB, NB)]
                )
                xt_tiles[g] = t0

            # ---- hh-gate group GEMM (xp_h staged two groups ahead) ----
            xp_tiles = {}
            hh_ps = {}

            def emit_xp_alloc(g):
                if g >= n_groups:
                    return
                xp_tiles[g] = p_xp.tile(
                    [128, NMH, NB], BF16, name=f"xp{g}", tag="xp"
                )

            def emit_hh_mms(g, m):
                if g >= n_groups:
                    return
                ps = p_g.tile([128, NB], F32, name=f"gps{g}_{m}", tag="gps")
                hh_ps[(g, m)] = ps
                xt_t = xt_tiles[g]
                for k in range(KF):
                    nc.tensor.matmul(
                        out=ps[:, :],
                        lhsT=wk_sb[:, k, ds((2 * KH + m) * 128, 128)],
                        rhs=xt_t[:, k, :],
                        start=(k == 0),
                        stop=(k == KF - 1),
                    )

            def emit_hh_evac(g, m, engine):
                if g >= n_groups:
                    return
                ps = hh_ps.pop((g, m))
                dst = xp_tiles[g][:, m, :]
                b_ap = biash_sb[:, ds(m, 1)]
                if engine == "act":
                    nc.scalar.activation(dst, ps[:, :], AF.Identity, bias=b_ap)
                else:
                    nc.vector.tensor_scalar_add(out=dst, in0=ps[:, :], scalar1=b_ap)

            # ---- per-step z/r PSUM bank seeding: bias inject + x-proj fill ----
            banks = {}

            def emit_inject_fill(t):
                if t >= n_steps:
                    return
                g, tau = divmod(t, GROUP)
                z_ps = p_z.tile([128, KH * BS], F32, name=f"z{t}", tag="z")
                r_ps = p_r.tile([128, KH * BS], F32, name=f"r{t}", tag="r")
                # hr lives in two banks (halves) so the tail's t1 can start on
                # half A while the PE still accumulates half B.
                hrA = p_hr.tile([128, HB], F32, name=f"hrA{t}", tag="hrA")
                hrB = p_hr.tile([128, HB], F32, name=f"hrB{t}", tag="hrB")
                nc.tensor.matmul(
                    out=z_ps[:, :], lhsT=bz_sb[:, :], rhs=ind_sb[:, :],
                    start=True, stop=False,
                )
                nc.tensor.matmul(
                    out=r_ps[:, :], lhsT=br_sb[:, :], rhs=ind_sb[:, :],
                    start=True, stop=False,
                )
                xt_t = xt_tiles[g]
                for gate, ps in ((0, z_ps), (1, r_ps)):
                    for m in range(KH):
                        for k in range(KF):
                            nc.tensor.matmul(
                                out=ps[:, ds(m * BS, BS)],
                                lhsT=wk_sb[:, k, ds((gate * KH + m) * 128, 128)],
                                rhs=xt_t[:, k, ds(tau * BS, BS)],
                                start=False, stop=False,
                                skip_group_check=True,
                            )
                # hr injections last: the hr banks are single-buffered, so
                # these wait on the previous step's t1 reads — by this point
                # in the PE stream those are long done.
                nc.tensor.matmul(
                    out=hrA[:, :], lhsT=brh_sb[:, :], rhs=ind_sb[:, 0:HB],
                    start=True, stop=False,
                )
                nc.tensor.matmul(
                    out=hrB[:, :], lhsT=brh_sb[:, :], rhs=ind_sb[:, HB:],
                    start=True, stop=False,
                )
                banks[t] = (z_ps, r_ps, hrA, hrB)

            # ---- prologue ----
            for g in range(min(3, n_groups)):
                emit_xt_dma(g)
            for g in range(min(2, n_groups)):
                emit_xp_alloc(g)
                for m in range(NMH):
                    emit_hh_mms(g, m)
                    emit_hh_evac(g, m, "act" if m % 2 == 0 else "dve")
            emit_inject_fill(0)

            h_prev = p_h.tile([128, KH * BS], BF16, name="h_init", tag="h")
            nc.vector.memset(h_prev[:, :], 0.0)

            # ---- main loop ----
            for t in range(n_steps):
                g, tau = divmod(t, GROUP)
                z_ps, r_ps, hrA, hrB = banks.pop(t)

                # recurrence matmuls: r gate, then z, then hr.  Within each
                # gate k=0,1 (first half of h) runs before k=2,3 so the first
                # matmuls start as soon as the previous chain's half lands.
                def rec_gate(ps, gate):
                    for kk in (0, 2):
                        for m in range(KH):
                            for k in (kk, kk + 1):
                                last = kk == 2 and m == KH - 1 and k == KH - 1
                                nc.tensor.matmul(
                                    out=ps[:, ds(m * BS, BS)],
                                    lhsT=wr_sb[:, k, ds((gate * KH + m) * 128, 128)],
                                    rhs=h_prev[:, ds(k * BS, BS)],
                                    start=False, stop=last,
                                    skip_group_check=not last,
                                )

                rec_gate(r_ps, 1)
                rsig = p_ew.tile([128, KH * BS], BF16, name=f"rs{t}", tag="rsig")
                nc.scalar.activation(rsig[:, :], r_ps[:, :], AF.Sigmoid)

                rec_gate(z_ps, 0)
                zsig = p_ew.tile([128, KH * BS], BF16, name=f"zs{t}", tag="zsig")
                nc.scalar.activation(zsig[:, :], z_ps[:, :], AF.Sigmoid)
                zc = p_ew.tile([128, KH * BS], BF16, name=f"zc{t}", tag="zc")
                nc.gpsimd.tensor_scalar(
                    out=zc[:, :], in0=zsig[:, :], scalar1=-1.0, scalar2=1.0,
                    op0=ALU.mult, op1=ALU.add,
                )
                a_t = p_ew.tile([128, KH * BS], BF16, name=f"a{t}", tag="a")
                nc.gpsimd.tensor_mul(a_t[:, :], zsig[:, :], h_prev[:, :])

                # hr: full h is available by now; run bank A's chunks first
                # and stop it so t1 half0 starts while bank B accumulates.
                for mm_lo, bank in ((0, hrA), (2, hrB)):
                    for m in (mm_lo, mm_lo + 1):
                        for k in range(KH):
                            last = m == mm_lo + 1 and k == KH - 1
                            nc.tensor.matmul(
                                out=bank[:, ds((m - mm_lo) * BS, BS)],
                                lhsT=wr_sb[:, k, ds((2 * KH + m) * 128, 128)],
                                rhs=h_prev[:, ds(k * BS, BS)],
                                start=False, stop=last,
                                skip_group_check=not last,
                            )

                # candidate tail: t1 = hr*r, t2 = t1+xp_h, hh = tanh(t2),
                # b = hh*zc, h = a+b.  Halves pipelined; DVE FIFO ordered to
                # avoid head-of-line blocking on the ACT tanh.
                xp_g = xp_tiles[g]
                t1h, t2h = [], []
                for half, hr_bank in ((0, hrA), (1, hrB)):
                    sl = ds(half * HB, HB)
                    t1 = p_ew.tile([128, HB], F32, name=f"t1_{t}_{half}", tag=f"t1{half}")
                    nc.vector.tensor_tensor(
                        out=t1[:, :], in0=hr_bank[:, :], in1=rsig[:, sl], op=ALU.mult
                    )
                    t2 = p_ew.tile([128, 2, BS], F32, name=f"t2_{t}_{half}", tag=f"t2{half}")
                    nc.vector.tensor_tensor(
                        out=t2[:, :, :],
                        in0=t1[:, :].rearrange("p (m b) -> p m b", b=BS),
                        in1=xp_g[:, ds(2 * half, 2), ds(tau * BS, BS)],
                        op=ALU.add,
                    )
                    t1h.append(t1)
                    t2h.append(t2)
                hhh = []
                for half in range(2):
                    hh = p_ew.tile([128, HB], BF16, name=f"hh{t}_{half}", tag=f"hh{half}")
                    nc.scalar.activation(
                        hh[:, :], t2h[half][:, :, :].rearrange("p m b -> p (m b)"),
                        AF.Tanh,
                    )
                    hhh.append(hh)
                h_new = p_h.tile([128, KH * BS], BF16, name=f"h{t}", tag="h")
                for half in range(2):
                    sl = ds(half * HB, HB)
                    b_t = p_ew.tile([128, HB], BF16, name=f"b{t}_{half}", tag=f"b{half}")
                    nc.vector.tensor_tensor(
                        out=b_t[:, :], in0=hhh[half][:, :], in1=zc[:, sl], op=ALU.mult
                    )
                    nc.vector.tensor_tensor(
                        out=h_new[:, sl], in0=a_t[:, sl], in1=b_t[:, :], op=ALU.add
                    )
                h_prev = h_new

                # next step's bank seeding + amortized hh GEMM + DMA
                if tau == 0:
                    emit_xt_dma(g + 3)
                    emit_xp_alloc(g + 2)
                emit_inject_fill(t + 1)
                if tau % 2 == 0:
                    emit_hh_mms(g + 2, tau // 2)
                else:
                    emit_hh_evac(g + 2, tau // 2, "act" if tau % 4 == 1 else "dve")

            # ---- dense head: out = h_last @ dense_w + dense_b ----
            d_ps = p_g.tile([1, BS], F32, name="dense_ps", tag="gps")
            for k in range(KH):
                nc.tensor.matmul(
                    out=d_ps[0:1, :],
                    lhsT=wd_sb[:, ds(k, 1)],
                    rhs=h_prev[:, ds(k * BS, BS)],
                    start=(k == 0),
                    stop=(k == KH - 1),
                )
            out_sb = p_const.tile([1, BS], F32)
            nc.scalar.activation(
                out_sb[0:1, :], d_ps[0:1, :], AF.Identity, bias=db_sb[0:1, 0:1]
            )
            nc.sync.dma_start(out=out.ap(), in_=out_sb[0:1, :])

    nc.finalize()
    return nc


def prep_inputs(x, kernel, rkernel, bias_i, bias_r, dense_w, dense_b, n_steps=T):
    """Host-side shard + layout prep. Returns in_maps for run_bass_kernel_spmd."""
    x = np.asarray(x, dtype=np.float32)
    kernel = np.asarray(kernel, dtype=np.float32)
    rkernel = np.asarray(rkernel, dtype=np.float32)
    bias_i = np.asarray(bias_i, dtype=np.float32)
    bias_r = np.asarray(bias_r, dtype=np.float32)
    dense_w = np.asarray(dense_w, dtype=np.float32)
    dense_b = np.asarray(dense_b, dtype=np.float32)

    wk_h = np.ascontiguousarray(kernel.astype(NP_BF16))
    wr_h = np.ascontiguousarray(rkernel.astype(NP_BF16))
    comb = bias_i[: 2 * H] + bias_r[: 2 * H]

    def pad_bias(v):
        m = np.zeros((128, 128), dtype=NP_BF16)
        m[:KH] = v.reshape(KH, 128).astype(NP_BF16)
        return np.ascontiguousarray(m)

    bz_h = pad_bias(comb[:H])
    br_h = pad_bias(comb[H:])
    brh_h = pad_bias(bias_r[2 * H:])
    ind_h = np.zeros((128, KH * BS), dtype=NP_BF16)
    for j in range(KH):
        ind_h[j, j * BS:(j + 1) * BS] = 1
    biash_h = np.ascontiguousarray(
        bias_i[2 * H:].reshape(NMH, 128).T.astype(np.float32)
    )
    wd_h = np.ascontiguousarray(dense_w.reshape(KH, 128).T.astype(NP_BF16))
    db_h = dense_b.reshape(1, 1).astype(np.float32)

    in_maps = []
    for c in range(NCORES):
        xs = x[c * BS:(c + 1) * BS]                       # [BS, T, F]
        xT_h = np.ascontiguousarray(
            xs.transpose(2, 1, 0).reshape(F, T * BS).astype(NP_BF16)
        )
        in_maps.append(
            {
                "xT": xT_h,
                "wk": wk_h,
                "wr": wr_h,
                "bz_l": bz_h,
                "br_l": br_h,
                "brh_l": brh_h,
                "ind": ind_h,
                "biash": biash_h,
                "wd": wd_h,
                "db": db_h,
            }
        )
    return in_maps


def kernel(x, kernel, rkernel, bias_i, bias_r, dense_w, dense_b):
    nc = build_program()
    in_maps = prep_inputs(x, kernel, rkernel, bias_i, bias_r, dense_w, dense_b)
    res = run_bass_kernel_spmd(nc, in_maps, list(range(NCORES)))
    outs = [res.results[i]["out"].reshape(BS, 1) for i in range(NCORES)]
    return np.concatenate(outs, axis=0).astype(np.float32)


# revision 40
# speedup vs baseline: 1.0383x; 1.0106x over previous
"""Trainium2 Bass kernel for a Keras-style GRU (reset_after=True) + Dense(1) head.

Reference computation (per batch row):
    x_proj = x @ kernel + bias_i                      # [T, 3H]
    per step t:  hp = h @ rkernel + bias_r            # [3H]
        z  = sigmoid(xp[:H]      + hp[:H])
        r  = sigmoid(xp[H:2H]    + hp[H:2H])
        hh = tanh   (xp[2H:]     + r * hp[2H:])
        h  = z * h + (1 - z) * hh
    out = h_last @ dense_w + dense_b                  # [1]

Strategy (8 NeuronCores, data-parallel over batch, 64 rows/core):
  - Everything lives in a [gate/H on partitions, batch on free] layout so the
    recurrence needs no transposes.  x is transposed/cast to bf16 on the host.
  - z/r gates: the per-step input projection is accumulated DIRECTLY into the
    recurrence PSUM bank one step ahead: a K=4 indicator matmul injects the
    bias (start=True opens the bank), 16 N=64 matmuls add x_t @ wk, and the
    next step's recurrence matmuls accumulate h @ rkernel on top.  No
    identity-injection matmuls, no SBUF staging, no evacuations for z/r.
  - hh gate: x @ wk staged group-wise (8 steps, N=512 GEMM) two groups ahead
    since xp_h sits outside the r* product; bias_r[2H:] injected per step via
    the K=4 indicator matmul.
  - Recurrence matmuls run r-gate first, then z, then hr, so sigmoid(r)/
    sigmoid(z)/zc/a all execute under the matmul shadow; only the candidate
    tail (t1 -> t2 -> tanh -> b -> h) trails each step, split in halves and
    pipelined across DVE/ACT.
"""

import os
import sys

sys.path.insert(0, "/opt/trn_rl_repo")

import numpy as np
import ml_dtypes

import concourse.bass as bass
import concourse.mybir as mybir
import concourse.tile as tile
from concourse import bacc
from concourse.bass import ds
from concourse.bass_utils import run_bass_kernel_spmd

BF16 = mybir.dt.bfloat16
F32 = mybir.dt.float32
NP_BF16 = ml_dtypes.bfloat16

NCORES = 8
B, T, F, H = 512, 128, 512, 512
BS = B // NCORES          # 64 batch rows per core
G3 = 3 * H                # 1536
KF = F // 128             # 4 contraction chunks for x @ kernel
KH = H // 128             # 4 contraction chunks for h @ rkernel
NMH = KH                  # 4 hh-gate chunks of 128
GROUP = 8                 # timesteps per hh GEMM group
NB = GROUP * BS           # 512 free columns per hh GEMM group
HB = KH * BS // 2         # 128: half of the hidden free dim
AF = mybir.ActivationFunctionType
ALU = mybir.AluOpType


def build_program(n_steps=T):
    """Emit the full Bass/Tile program for one core."""
    n_groups = (n_steps + GROUP - 1) // GROUP
    nc = bacc.Bacc()

    # ---- DRAM parameters (per-core shapes; host pre-arranges layouts) ----
    xT = nc.declare_dram_parameter("xT", [F, T * BS], BF16, isOutput=False)
    wk = nc.declare_dram_parameter("wk", [F, G3], BF16, isOutput=False)
    wr = nc.declare_dram_parameter("wr", [H, G3], BF16, isOutput=False)
    # bias-injection lhsT tiles: row j holds the bias for gate chunk j; rows
    # 4..127 are zero so the matmul keeps the PE's (128,128) tile config (a
    # K=4 tile forces a config switch costing ~200ns on the next matmul).
    bz_l = nc.declare_dram_parameter("bz_l", [128, 128], BF16, isOutput=False)
    br_l = nc.declare_dram_parameter("br_l", [128, 128], BF16, isOutput=False)
    brh_l = nc.declare_dram_parameter("brh_l", [128, 128], BF16, isOutput=False)
    ind = nc.declare_dram_parameter("ind", [128, KH * BS], BF16, isOutput=False)
    # per-partition ACT bias for the hh GEMM evacuation (bias_i[2H:])
    biash = nc.declare_dram_parameter("biash", [128, NMH], F32, isOutput=False)
    wd = nc.declare_dram_parameter("wd", [128, KH], BF16, isOutput=False)
    db = nc.declare_dram_parameter("db", [1, 1], F32, isOutput=False)
    out = nc.declare_dram_parameter("out", [1, BS], F32, isOutput=True)

    xT_v = xT.ap().rearrange("(k p) n -> p k n", p=128)    # [128, KF, T*BS]
    wk_v = wk.ap().rearrange("(k p) g -> p k g", p=128)    # [128, KF, G3]
    wr_v = wr.ap().rearrange("(k p) g -> p k g", p=128)    # [128, KH, G3]

    with tile.TileContext(nc) as tc:
        with (
            tc.tile_pool(name="const", bufs=1) as p_const,
            tc.tile_pool(name="xt", bufs=4) as p_xt,
            tc.tile_pool(name="xp", bufs=3) as p_xp,
            tc.tile_pool(name="h", bufs=3) as p_h,
            tc.tile_pool(name="ew", bufs=2) as p_ew,
            tc.tile_pool(name="zps", bufs=2, space="PSUM") as p_z,
            tc.tile_pool(name="rps", bufs=2, space="PSUM") as p_r,
            tc.tile_pool(name="hrps", bufs=1, space="PSUM") as p_hr,
            tc.tile_pool(name="gps", bufs=2, space="PSUM") as p_g,
        ):
            # ---- resident constants ----
            # DMA queue order matters for the prologue: the tiny injection
            # tensors and group 0 of x go first so the step-0 bank seeding
            # and fills can start while the big weight DMAs stream in; wk/wr
            # are split in k-halves so dependent matmuls start on the first
            # half.
            bz_sb = p_const.tile([128, 128], BF16)
            nc.sync.dma_start(out=bz_sb[:, :], in_=bz_l.ap())
            br_sb = p_const.tile([128, 128], BF16)
            nc.sync.dma_start(out=br_sb[:, :], in_=br_l.ap())
            brh_sb = p_const.tile([128, 128], BF16)
            nc.sync.dma_start(out=brh_sb[:, :], in_=brh_l.ap())
            ind_sb = p_const.tile([128, KH * BS], BF16)
            nc.sync.dma_start(out=ind_sb[:, :], in_=ind.ap())

            # ---- xT group DMA ----
            xt_tiles = {}

            def emit_xt_dma(g):
                if g >= n_groups:
                    return
                t0 = p_xt.tile([128, KF, NB], BF16, name=f"xt{g}", tag="xt")
                nc.sync.dma_start(
                    out=t0[:, :, :], in_=xT_v[:, :, ds(g * NB, NB)]
                )
                xt_tiles[g] = t0

            emit_xt_dma(0)
            wk_sb = p_const.tile([128, KF, G3], BF16)
            nc.sync.dma_start(out=wk_sb[:, 0:2, :], in_=wk_v[:, 0:2, :])
            nc.sync.dma_start(out=wk_sb[:, 2:4, :], in_=wk_v[:, 2:4, :])
            wr_sb = p_const.tile([128, KH, G3], BF16)
            nc.sync.dma_start(out=wr_sb[:, 0:2, :], in_=wr_v[:, 0:2, :])
            nc.sync.dma_start(out=wr_sb[:, 2:4, :], in_=wr_v[:, 2:4, :])
            biash_sb = p_const.tile([128, NMH], F32)
            nc.sync.dma_start(out=biash_sb[:, :], in_=biash.ap())
            wd_sb = p_const.tile([128, KH], BF16)
            nc.sync.dma_start(out=wd_sb[:, :], in_=wd.ap())
            db_sb = p_const.tile([1, 1], F32)
            nc.sync.dma_start(out=db_sb[:, :], in_=db.ap())

            # ---- hh-gate group GEMM (xp_h staged two groups ahead) ----
            xp_tiles = {}
            hh_ps = {}

            def emit_xp_alloc(g):
                if g >= n_groups:
                    return
                xp_tiles[g] = p_xp.tile(
                    [128, NMH, NB], BF16, name=f"xp{g}", tag="xp"
                )

            def emit_hh_mms(g, m):
                if g >= n_groups:
                    return
                ps = p_g.tile([128, NB], F32, name=f"gps{g}_{m}", tag="gps")
                hh_ps[(g, m)] = ps
                xt_t = xt_tiles[g]
                for k in range(KF):
                    nc.tensor.matmul(
                        out=ps[:, :],
                        lhsT=wk_sb[:, k, ds((2 * KH + m) * 128, 128)],
                        rhs=xt_t[:, k, :],
                        start=(k == 0),
                        stop=(k == KF - 1),
                    )

            def emit_hh_evac(g, m, engine):
                if g >= n_groups:
                    return
                ps = hh_ps.pop((g, m))
                dst = xp_tiles[g][:, m, :]
                b_ap = biash_sb[:, ds(m, 1)]
                if engine == "act":
                    nc.scalar.activation(dst, ps[:, :], AF.Identity, bias=b_ap)
                else:
                    nc.vector.tensor_scalar_add(out=dst, in0=ps[:, :], scalar1=b_ap)

            # ---- per-step z/r PSUM bank seeding: bias inject + x-proj fill ----
            banks = {}

            def emit_inject_fill(t):
                if t >= n_steps:
                    return
                g, tau = divmod(t, GROUP)
                z_ps = p_z.tile([128, KH * BS], F32, name=f"z{t}", tag="z")
                r_ps = p_r.tile([128, KH * BS], F32, name=f"r{t}", tag="r")
                # hr lives in two banks (halves) so the tail's t1 can start on
                # half A while the PE still accumulates half B.
                hrA = p_hr.tile([128, HB], F32, name=f"hrA{t}", tag="hrA")
                hrB = p_hr.tile([128, HB], F32, name=f"hrB{t}", tag="hrB")
                nc.tensor.matmul(
                    out=z_ps[:, :], lhsT=bz_sb[:, :], rhs=ind_sb[:, :],
                    start=True, stop=False,
                )
                nc.tensor.matmul(
                    out=r_ps[:, :], lhsT=br_sb[:, :], rhs=ind_sb[:, :],
                    start=True, stop=False,
                )
                xt_t = xt_tiles[g]
                for gate, ps in ((0, z_ps), (1, r_ps)):
                    for m in range(KH):
                        for k in range(KF):
                            nc.tensor.matmul(
                                out=ps[:, ds(m * BS, BS)],
                                lhsT=wk_sb[:, k, ds((gate * KH + m) * 128, 128)],
                                rhs=xt_t[:, k, ds(tau * BS, BS)],
                                start=False, stop=False,
                                skip_group_check=True,
                            )
                # hr injections last: the hr banks are single-buffered, so
                # these wait on the previous step's t1 reads — by this point
                # in the PE stream those are long done.
                nc.tensor.matmul(
                    out=hrA[:, :], lhsT=brh_sb[:, :], rhs=ind_sb[:, 0:HB],
                    start=True, stop=False,
                )
                nc.tensor.matmul(
                    out=hrB[:, :], lhsT=brh_sb[:, :], rhs=ind_sb[:, HB:],
                    start=True, stop=False,
                )
                banks[t] = (z_ps, r_ps, hrA, hrB)

            # ---- prologue (xt group 0 was DMA'd with the consts) ----
            for g in range(1, min(3, n_groups)):
                emit_xt_dma(g)
            for g in range(min(2, n_groups)):
                emit_xp_alloc(g)
                for m in range(NMH):
                    emit_hh_mms(g, m)
                    emit_hh_evac(g, m, "act" if m % 2 == 0 else "dve")
            emit_inject_fill(0)

            h_prev = p_h.tile([128, KH * BS], BF16, name="h_init", tag="h")
            nc.vector.memset(h_prev[:, :], 0.0)

            # ---- main loop ----
            for t in range(n_steps):
                g, tau = divmod(t, GROUP)
                z_ps, r_ps, hrA, hrB = banks.pop(t)

                # recurrence matmuls: r gate, then z, then hr.  Within each
                # gate k=0,1 (first half of h) runs before k=2,3 so the first
                # matmuls start as soon as the previous chain's half lands.
                def rec_gate(ps, gate):
                    for kk in (0, 2):
                        for m in range(KH):
                            for k in (kk, kk + 1):
                                last = kk == 2 and m == KH - 1 and k == KH - 1
                                nc.tensor.matmul(
                                    out=ps[:, ds(m * BS, BS)],
                                    lhsT=wr_sb[:, k, ds((gate * KH + m) * 128, 128)],
                                    rhs=h_prev[:, ds(k * BS, BS)],
                                    start=False, stop=last,
                                    skip_group_check=not last,
                                )

                rec_gate(r_ps, 1)
                rsig = p_ew.tile([128, KH * BS], BF16, name=f"rs{t}", tag="rsig")
                nc.scalar.activation(rsig[:, :], r_ps[:, :], AF.Sigmoid)

                rec_gate(z_ps, 0)
                zsig = p_ew.tile([128, KH * BS], BF16, name=f"zs{t}", tag="zsig")
                nc.scalar.activation(zsig[:, :], z_ps[:, :], AF.Sigmoid)
                zc = p_ew.tile([128, KH * BS], BF16, name=f"zc{t}", tag="zc")
                nc.gpsimd.tensor_scalar(
                    out=zc[:, :], in0=zsig[:, :], scalar1=-1.0, scalar2=1.0,
                    op0=ALU.mult, op1=ALU.add,
                )
                a_t = p_ew.tile([128, KH * BS], BF16, name=f"a{t}", tag="a")
                nc.gpsimd.tensor_mul(a_t[:, :], zsig[:, :], h_prev[:, :])

                # hr: full h is available by now; run bank A's chunks first
                # and stop it so t1 half0 starts while bank B accumulates.
                for mm_lo, bank in ((0, hrA), (2, hrB)):
                    for m in (mm_lo, mm_lo + 1):
                        for k in range(KH):
                            last = m == mm_lo + 1 and k == KH - 1
                            nc.tensor.matmul(
                                out=bank[:, ds((m - mm_lo) * BS, BS)],
                                lhsT=wr_sb[:, k, ds((2 * KH + m) * 128, 128)],
                                rhs=h_prev[:, ds(k * BS, BS)],
                                start=False, stop=last,
                                skip_group_check=not last,
                            )

                # candidate tail: t1 = hr*r, t2 = t1+xp_h, hh = tanh(t2),
                # b = hh*zc, h = a+b.  Halves pipelined; DVE FIFO ordered to
                # avoid head-of-line blocking on the ACT tanh.
                xp_g = xp_tiles[g]
                t1h, t2h = [], []
                for half, hr_bank in ((0, hrA), (1, hrB)):
                    sl = ds(half * HB, HB)
                    t1 = p_ew.tile([128, HB], F32, name=f"t1_{t}_{half}", tag=f"t1{half}")
                    nc.vector.tensor_tensor(
                        out=t1[:, :], in0=hr_bank[:, :], in1=rsig[:, sl], op=ALU.mult
                    )
                    t2 = p_ew.tile([128, 2, BS], F32, name=f"t2_{t}_{half}", tag=f"t2{half}")
                    nc.vector.tensor_tensor(
                        out=t2[:, :, :],
                        in0=t1[:, :].rearrange("p (m b) -> p m b", b=BS),
                        in1=xp_g[:, ds(2 * half, 2), ds(tau * BS, BS)],
                        op=ALU.add,
                    )
                    t1h.append(t1)
                    t2h.append(t2)
                hhh = []
                for half in range(2):
                    hh = p_ew.tile([128, HB], BF16, name=f"hh{t}_{half}", tag=f"hh{half}")
                    nc.scalar.activation(
                        hh[:, :], t2h[half][:, :, :].rearrange("p m b -> p (m b)"),
                        AF.Tanh,
                    )
                    hhh.append(hh)
                h_new = p_h.tile([128, KH * BS], BF16, name=f"h{t}", tag="h")
                for half in range(2):
                    sl = ds(half * HB, HB)
                    b_t = p_ew.tile([128, HB], BF16, name=f"b{t}_{half}", tag=f"b{half}")
                    nc.vector.tensor_tensor(
                        out=b_t[:, :], in0=hhh[half][:, :], in1=zc[:, sl], op=ALU.mult
                    )
                    nc.vector.tensor_tensor(
                        out=h_new[:, sl], in0=a_t[:, sl], in1=b_t[:, :], op=ALU.add
                    )
                h_prev = h_new

                # next step's bank seeding + amortized hh GEMM + DMA
                if tau == 0:
                    emit_xt_dma(g + 3)
                    emit_xp_alloc(g + 2)
                emit_inject_fill(t + 1)
                if tau % 2 == 0:
                    emit_hh_mms(g + 2, tau // 2)
                else:
                    emit_hh_evac(g + 2, tau // 2, "act" if tau % 4 == 1 else "dve")

            # ---- dense head: out = h_last @ dense_w + dense_b ----
            d_ps = p_g.tile([1, BS], F32, name="dense_ps", tag="gps")
            for k in range(KH):
                nc.tensor.matmul(
                    out=d_ps[0:1, :],
                    lhsT=wd_sb[:, ds(k, 1)],
                    rhs=h_prev[:, ds(k * BS, BS)],
                    start=(k == 0),
                    stop=(k == KH - 1),
                )
            out_sb = p_const.tile([1, BS], F32)
            nc.scalar.activation(
                out_sb[0:1, :], d_ps[0:1, :], AF.Identity, bias=db_sb[0:1, 0:1]
            )
            nc.sync.dma_start(out=out.ap(), in_=out_sb[0:1, :])

    nc.finalize()
    return nc


def prep_inputs(x, kernel, rkernel, bias_i, bias_r, dense_w, dense_b, n_steps=T):
    """Host-side shard + layout prep. Returns in_maps for run_bass_kernel_spmd."""
    x = np.asarray(x, dtype=np.float32)
    kernel = np.asarray(kernel, dtype=np.float32)
    rkernel = np.asarray(rkernel, dtype=np.float32)
    bias_i = np.asarray(bias_i, dtype=np.float32)
    bias_r = np.asarray(bias_r, dtype=np.float32)
    dense_w = np.asarray(dense_w, dtype=np.float32)
    dense_b = np.asarray(dense_b, dtype=np.float32)

    wk_h = np.ascontiguousarray(kernel.astype(NP_BF16))
    wr_h = np.ascontiguousarray(rkernel.astype(NP_BF16))
    comb = bias_i[: 2 * H] + bias_r[: 2 * H]

    def pad_bias(v):
        m = np.zeros((128, 128), dtype=NP_BF16)
        m[:KH] = v.reshape(KH, 128).astype(NP_BF16)
        return np.ascontiguousarray(m)

    bz_h = pad_bias(comb[:H])
    br_h = pad_bias(comb[H:])
    brh_h = pad_bias(bias_r[2 * H:])
    ind_h = np.zeros((128, KH * BS), dtype=NP_BF16)
    for j in range(KH):
        ind_h[j, j * BS:(j + 1) * BS] = 1
    biash_h = np.ascontiguousarray(
        bias_i[2 * H:].reshape(NMH, 128).T.astype(np.float32)
    )
    wd_h = np.ascontiguousarray(dense_w.reshape(KH, 128).T.astype(NP_BF16))
    db_h = dense_b.reshape(1, 1).astype(np.float32)

    in_maps = []
    for c in range(NCORES):
        xs = x[c * BS:(c + 1) * BS]                       # [BS, T, F]
        xT_h = np.ascontiguousarray(
            xs.transpose(2, 1, 0).reshape(F, T * BS).astype(NP_BF16)
        )
        in_maps.append(
            {
                "xT": xT_h,
                "wk": wk_h,
                "wr": wr_h,
                "bz_l": bz_h,
                "br_l": br_h,
                "brh_l": brh_h,
                "ind": ind_h,
                "biash": biash_h,
                "wd": wd_h,
                "db": db_h,
            }
        )
    return in_maps


def kernel(x, kernel, rkernel, bias_i, bias_r, dense_w, dense_b):
    nc = build_program()
    in_maps = prep_inputs(x, kernel, rkernel, bias_i, bias_r, dense_w, dense_b)
    res = run_bass_kernel_spmd(nc, in_maps, list(range(NCORES)))
    outs = [res.results[i]["out"].reshape(BS, 1) for i in range(NCORES)]
    return np.concatenate(outs, axis=0).astype(np.float32)
